# revision 1
# baseline (speedup 1.0000x reference)
"""Multi-scale deformable attention (MSDeformAttn) Trainium2 Bass kernel.

Sharding: 16 (batch, head)-slices across 8 cores -> each core owns one batch
element b = core//4 and two heads m0 = 2*(core%4), m0+1.  Each core:
  A) transposes query/x chunks on PE, projects offsets/attn-logits/value
  B) writes padded per-head value to DRAM, builds a flat "quad table" QT
     where row u holds the 4 bilinear corner rows of quad position u
  C) computes sampling locations and bilinear*attention corner weights (DVE)
  D) rearranges int16 quad indices into the SWDGE wrapped index layout
  E) dma_gather: one 512B descriptor fetches all 4 corners of one
     (query, level, point) sample; DVE weighted multiply + segmented reduce
  F) PE projects the per-head sampled outputs through this core's Wo rows
Host sums the 4 per-core partials per batch element and adds bo.
"""

import contextlib

import numpy as np

import concourse.bass as bass
import concourse.bacc as bacc
import concourse.mybir as mybir
import concourse.tile as tile
from concourse.bass_utils import run_bass_kernel_spmd
from concourse.masks import make_identity

AL = mybir.AluOpType
DT = mybir.dt
AF = mybir.ActivationFunctionType

# problem constants
B, LQ, DM, M, L, P, D = 2, 12240, 256, 8, 4, 4, 32
SHAPES = [(96, 96), (48, 48), (24, 24), (12, 12)]
STARTS = [0, 9216, 11520, 12096]
LQP = 12288         # LQ padded to a multiple of 128 (host zero-pads inputs)
NCH = 96            # query chunks of 128
G = 4               # chunks per gather group
NCG = NCH // G      # gather groups
NI = G * 16 * 128   # indices per gather call

# per-level flat quad-table geometry
PADL = [2 * w + 4 for (h, w) in SHAPES]
TAILL = [w + 4 for (h, w) in SHAPES]
NL = [h * w + p + t for (h, w), p, t in zip(SHAPES, PADL, TAILL)]
U0 = [0, 0, 0, 0]
for _l in range(1, 4):
    U0[_l] = U0[_l - 1] + NL[_l - 1]
QTN = U0[3] + NL[3]
VP_LEAD = 200
VP_ROWS = VP_LEAD + LQP + 80

_NC_CACHE = {}


def _consts_array():
    """9 rows of 16 = (level, point)-flattened per-level constants."""
    W = [w for (h, w) in SHAPES]
    H = [h for (h, w) in SHAPES]

    def per_lp(vals):
        return np.repeat(np.asarray(vals, np.float32), P)

    return np.stack([
        per_lp(W),                                    # 0 Wc
        per_lp(H),                                    # 1 Hc
        per_lp([w - 1 for w in W]),                   # 2 Wm1
        per_lp([w - 2 for w in W]),                   # 3 Wm2
        per_lp([h - 1 for h in H]),                   # 4 Hm1
        per_lp([h - 2 for h in H]),                   # 5 Hm2
        per_lp(U0),                                   # 6 Ulo
        per_lp([U0[l] + NL[l] - 1 for l in range(4)]),  # 7 Uhi
        per_lp([U0[l] + PADL[l] for l in range(4)]),  # 8 Upad
    ])


def bap(t, dims, off=0):
    a = t if isinstance(t, bass.AP) else t[:]
    return bass.AP(tensor=a.tensor, offset=a.offset + off, ap=dims)


def split_waits(nc, maxw=1):
    """This walrus encodes at most one semaphore wait per instruction; hoist
    excess waits onto same-engine nops placed immediately before."""
    n = 0
    for f in nc.m.functions:
        for bb in f.blocks:
            nl = []
            for ins in bb.instructions:
                si = ins.sync_info
                if si is not None and si.on_wait and len(si.on_wait) > maxw:
                    ws = list(si.on_wait)
                    exc = ws[:-maxw]
                    si.on_wait = ws[-maxw:]
                    for i in range(0, len(exc), maxw):
                        nop = mybir.InstNoOp(
                            name=f"{ins.name}-ws{i}", engine=ins.engine)
                        nop.sync_info = mybir.SyncInfo(
                            on_wait=exc[i:i + maxw], on_update=[])
                        nl.append(nop)
                        n += 1
                nl.append(ins)
            bb.instructions = nl
    return n


def build_nc(do_split=True):
    nc = bacc.Bacc("TRN2", target_bir_lowering=False, debug=False)
    f32, i16 = DT.float32, DT.int16

    x_d = nc.declare_dram_parameter("x", [LQP, DM], f32, isOutput=False)
    q_d = nc.declare_dram_parameter("q", [LQP, DM], f32, isOutput=False)
    ref_d = nc.declare_dram_parameter("ref", [LQP, 8], f32, isOutput=False)
    wv_d = nc.declare_dram_parameter("wv", [DM, 64], f32, isOutput=False)
    ws_d = nc.declare_dram_parameter("ws", [DM, 64], f32, isOutput=False)
    wa_d = nc.declare_dram_parameter("wa", [DM, 32], f32, isOutput=False)
    wo_d = nc.declare_dram_parameter("wo", [64, DM], f32, isOutput=False)
    bs_d = nc.declare_dram_parameter("bs", [64], f32, isOutput=False)
    bv_d = nc.declare_dram_parameter("bv", [64], f32, isOutput=False)
    ba_d = nc.declare_dram_parameter("ba", [32], f32, isOutput=False)
    cst_d = nc.declare_dram_parameter("consts", [9, 16], f32, isOutput=False)
    out_d = nc.declare_dram_parameter("out", [LQP, DM], f32, isOutput=True)

    offs_dram = nc.dram_tensor("offs_dram", [LQP, 64], f32)
    vp = [nc.dram_tensor(f"vp{h}", [VP_ROWS, D], f32) for h in range(2)]
    qt = [nc.dram_tensor(f"qt{h}", [QTN, 128], f32) for h in range(2)]

    with tile.TileContext(nc) as tc, contextlib.ExitStack() as ctx:
        const_p = ctx.enter_context(tc.tile_pool(name="const", bufs=1))
        ident = const_p.tile([128, 128], f32)
        make_identity(nc, ident[:])

        ws_t = const_p.tile([128, 2, 64], f32)
        nc.sync.dma_start(out=ws_t[:], in_=bap(ws_d, [[64, 128], [128 * 64, 2], [1, 64]]))
        wa_t = const_p.tile([128, 2, 32], f32)
        nc.sync.dma_start(out=wa_t[:], in_=bap(wa_d, [[32, 128], [128 * 32, 2], [1, 32]]))
        wv_t = const_p.tile([128, 2, 64], f32)
        nc.sync.dma_start(out=wv_t[:], in_=bap(wv_d, [[64, 128], [128 * 64, 2], [1, 64]]))
        wo_t = const_p.tile([64, DM], f32)
        nc.sync.dma_start(out=wo_t[:], in_=wo_d[:])

        # broadcast rows across partitions (SWDGE handles 0-step partition)
        cst_t = const_p.tile([128, 9, 16], f32)
        nc.gpsimd.dma_start(out=cst_t[:], in_=bap(cst_d, [[0, 128], [16, 9], [1, 16]]))
        bs_t = const_p.tile([128, 64], f32)
        nc.gpsimd.dma_start(out=bs_t[:], in_=bap(bs_d, [[0, 128], [1, 64]]))
        bv_t = const_p.tile([128, 64], f32)
        nc.gpsimd.dma_start(out=bv_t[:], in_=bap(bv_d, [[0, 128], [1, 64]]))
        ba_t = const_p.tile([128, 32], f32)
        nc.gpsimd.dma_start(out=ba_t[:], in_=bap(ba_d, [[0, 128], [1, 32]]))
        ref_t = const_p.tile([128, NCH, 8], f32)
        nc.sync.dma_start(
            out=ref_t[:], in_=bap(ref_d, [[8, 128], [128 * 8, NCH], [1, 8]]))

        def cview(row):  # const row -> [128, (0,NCH), 4, 4] broadcast view
            return bap(cst_t, [[144, 128], [0, NCH], [4, 4], [1, 4]], off=row * 16)

        def cview_flat(row):  # const row -> [128, (0,NCH), 16]
            return bap(cst_t, [[144, 128], [0, NCH], [1, 16]], off=row * 16)

        # persistent across phases
        big_p = ctx.enter_context(tc.tile_pool(name="big", bufs=1))
        logit_all = big_p.tile([128, NCH, 32], f32)
        sampled = [big_p.tile([128, NCH, D], f32, tag=f"sampled{_h}", name=f"sampled{_h}")
                   for _h in range(2)]

        # ---------------- phase A: projections ----------------
        with tc.tile_pool(name="phA", bufs=1) as apool, \
             tc.tile_pool(name="phA_s", bufs=3) as spool, \
             tc.tile_pool(name="phA_ps", bufs=2, space="PSUM") as pspool:
            offs_stage = apool.tile([128, NCH, 64], f32)
            val_all = apool.tile([128, NCH, 64], f32)
            for c in range(NCH):
                qtile = spool.tile([128, DM], f32, tag="ld")
                nc.sync.dma_start(out=qtile[:], in_=q_d[c * 128:(c + 1) * 128, :])
                xtile = spool.tile([128, DM], f32, tag="ld")
                nc.sync.dma_start(out=xtile[:], in_=x_d[c * 128:(c + 1) * 128, :])
                qT = spool.tile([128, 2, 128], f32, tag="qT")
                xT = spool.tile([128, 2, 128], f32, tag="xT")
                for t in range(2):
                    tp = pspool.tile([128, 128], f32, tag="tp")
                    nc.tensor.transpose(out=tp[:], in_=qtile[:, t * 128:(t + 1) * 128],
                                        identity=ident[:])
                    nc.scalar.copy(out=qT[:, t, :], in_=tp[:])
                    tp2 = pspool.tile([128, 128], f32, tag="tp")
                    nc.tensor.transpose(out=tp2[:], in_=xtile[:, t * 128:(t + 1) * 128],
                                        identity=ident[:])
                    nc.scalar.copy(out=xT[:, t, :], in_=tp2[:])
                po = pspool.tile([128, 64], f32, tag="po")
                pl = pspool.tile([128, 32], f32, tag="pl")
                pv = pspool.tile([128, 64], f32, tag="pv")
                for t in range(2):
                    nc.tensor.matmul(out=po[:], lhsT=qT[:, t, :], rhs=ws_t[:, t, :],
                                     start=(t == 0), stop=(t == 1))
                for t in range(2):
                    nc.tensor.matmul(out=pl[:], lhsT=qT[:, t, :], rhs=wa_t[:, t, :],
                                     start=(t == 0), stop=(t == 1))
                for t in range(2):
                    nc.tensor.matmul(out=pv[:], lhsT=xT[:, t, :], rhs=wv_t[:, t, :],
                                     start=(t == 0), stop=(t == 1))
                nc.scalar.copy(out=offs_stage[:, c, :], in_=po[:])
                nc.scalar.copy(out=logit_all[:, c, :], in_=pl[:])
                nc.scalar.copy(out=val_all[:, c, :], in_=pv[:])

            nc.vector.tensor_tensor(
                out=logit_all[:], in0=logit_all[:],
                in1=bap(ba_t, [[32, 128], [0, NCH], [1, 32]]), op=AL.add)
            nc.vector.tensor_tensor(
                out=val_all[:], in0=val_all[:],
                in1=bap(bv_t, [[64, 128], [0, NCH], [1, 64]]), op=AL.add)
            nc.sync.dma_start(
                out=bap(offs_dram, [[64, 128], [128 * 64, NCH], [1, 64]]),
                in_=offs_stage[:])

            # ---------------- phase B: padded value + quad tables ----
            zt = apool.tile([128, D], f32)
            nc.vector.memset(zt[:], 0.0)
            for h in range(2):
                nc.sync.dma_start(
                    out=bap(vp[h], [[D, 128], [128 * D, NCH], [1, D]], off=VP_LEAD * D),
                    in_=val_all[:, :, h * D:(h + 1) * D])
                nc.sync.dma_start(
                    out=bap(vp[h], [[D, 128], [1, D]]), in_=zt[0:128, :])
                nc.sync.dma_start(
                    out=bap(vp[h], [[D, VP_LEAD - 128], [1, D]], off=128 * D),
                    in_=zt[0:VP_LEAD - 128, :])
                nc.sync.dma_start(
                    out=bap(vp[h], [[D, 80], [1, D]], off=(VP_LEAD + LQP) * D),
                    in_=zt[0:80, :])
            for h in range(2):
                for l in range(4):
                    w_l = SHAPES[l][1]
                    base = VP_LEAD + STARTS[l] - PADL[l]
                    for pc, dlt in ((0, 0), (1, w_l)):
                        nc.sync.dma_start(
                            out=bap(qt[h], [[128, NL[l]], [1, 64]],
                                    off=U0[l] * 128 + pc * 64),
                            in_=bap(vp[h], [[D, NL[l]], [1, 64]],
                                    off=(base + dlt) * D))

        # ---------------- phases C-E per head ----------------
        idx_p = ctx.enter_context(tc.tile_pool(name="idx", bufs=1))
        for h in range(2):
            with tc.tile_pool(name="phC", bufs=1) as cpool:
                offs_h = cpool.tile([128, NCH, 32], f32, tag="offs")
                nc.sync.dma_start(
                    out=offs_h[:],
                    in_=bap(offs_dram, [[64, 128], [128 * 64, NCH], [1, 32]],
                            off=h * 32))

                w4 = idx_p.tile([128, NCH, 64], f32, tag="w4")
                u16 = idx_p.tile([128, NCH, 16], i16, tag="u16")

                wghts = []
                x0y0 = []
                for ax in range(2):  # 0=x, 1=y
                    Wrow = 0 if ax == 0 else 1
                    m1row = 2 if ax == 0 else 4
                    m2row = 3 if ax == 0 else 5
                    cp = cpool.tile([128, 16], f32, tag="cp")
                    nc.vector.tensor_scalar(
                        out=cp[:],
                        in0=bap(bs_t, [[64, 128], [2, 16]], off=h * 32 + ax),
                        scalar1=-0.5, scalar2=None, op0=AL.add)
                    X = cpool.tile([128, NCH, 16], f32, tag="tA")
                    nc.vector.tensor_tensor(
                        out=X[:],
                        in0=bap(offs_h, [[3072, 128], [32, NCH], [2, 16]], off=ax),
                        in1=bap(cp, [[16, 128], [0, NCH], [1, 16]]), op=AL.add)
                    xw = cpool.tile([128, NCH, 16], f32, tag="tB")
                    nc.vector.tensor_tensor(
                        out=bap(xw, [[1536, 128], [16, NCH], [4, 4], [1, 4]]),
                        in0=bap(ref_t, [[768, 128], [8, NCH], [2, 4], [0, 4]], off=ax),
                        in1=cview(Wrow), op=AL.mult)
                    nc.vector.tensor_tensor(out=X[:], in0=X[:], in1=xw[:], op=AL.add)
                    # exact floor: int16 roundtrip + fixup
                    ri = cpool.tile([128, NCH, 16], i16, tag="ri")
                    rf = cpool.tile([128, NCH, 16], f32, tag="tB")
                    nc.vector.tensor_copy(out=ri[:], in_=X[:])
                    nc.vector.tensor_copy(out=rf[:], in_=ri[:])
                    gt = cpool.tile([128, NCH, 16], f32, tag="tC")
                    nc.vector.tensor_tensor(out=gt[:], in0=rf[:], in1=X[:], op=AL.is_gt)
                    X0 = cpool.tile([128, NCH, 16], f32, tag=f"X0{ax}")
                    nc.vector.tensor_tensor(out=X0[:], in0=rf[:], in1=gt[:], op=AL.subtract)
                    fx = cpool.tile([128, NCH, 16], f32, tag="fx")
                    nc.vector.tensor_tensor(out=fx[:], in0=X[:], in1=X0[:], op=AL.subtract)
                    # valid masks for left/right (top/bottom) corners
                    vl = cpool.tile([128, NCH, 16], f32, tag="vl")
                    nc.vector.tensor_scalar(out=vl[:], in0=X0[:], scalar1=0.0,
                                            scalar2=None, op0=AL.is_ge)
                    t2 = cpool.tile([128, NCH, 16], f32, tag="tC")
                    nc.vector.tensor_tensor(out=t2[:], in0=X0[:], in1=cview_flat(m1row),
                                            op=AL.is_le)
                    nc.vector.tensor_tensor(out=vl[:], in0=vl[:], in1=t2[:], op=AL.mult)
                    vr = cpool.tile([128, NCH, 16], f32, tag="vr")
                    nc.vector.tensor_scalar(out=vr[:], in0=X0[:], scalar1=-1.0,
                                            scalar2=None, op0=AL.is_ge)
                    t3 = cpool.tile([128, NCH, 16], f32, tag="tC")
                    nc.vector.tensor_tensor(out=t3[:], in0=X0[:], in1=cview_flat(m2row),
                                            op=AL.is_le)
                    nc.vector.tensor_tensor(out=vr[:], in0=vr[:], in1=t3[:], op=AL.mult)
                    omf = cpool.tile([128, NCH, 16], f32, tag="tC")
                    nc.vector.tensor_scalar(out=omf[:], in0=fx[:], scalar1=-1.0,
                                            scalar2=1.0, op0=AL.mult, op1=AL.add)
                    wl = cpool.tile([128, NCH, 16], f32, tag=f"wl{ax}")
                    nc.vector.tensor_tensor(out=wl[:], in0=omf[:], in1=vl[:], op=AL.mult)
                    wr = cpool.tile([128, NCH, 16], f32, tag=f"wr{ax}")
                    nc.vector.tensor_tensor(out=wr[:], in0=fx[:], in1=vr[:], op=AL.mult)
                    wghts.append((wl, wr))
                    x0y0.append(X0)

                # u = clamp(y0*W + x0 + Upad, Ulo, Uhi) -> int16
                uf = cpool.tile([128, NCH, 16], f32, tag="tA")
                nc.vector.tensor_tensor(out=uf[:], in0=x0y0[1][:], in1=cview_flat(0),
                                        op=AL.mult)
                nc.vector.tensor_tensor(out=uf[:], in0=uf[:], in1=x0y0[0][:], op=AL.add)
                nc.vector.tensor_tensor(out=uf[:], in0=uf[:], in1=cview_flat(8), op=AL.add)
                nc.vector.tensor_tensor(out=uf[:], in0=uf[:], in1=cview_flat(6), op=AL.max)
                nc.vector.tensor_tensor(out=uf[:], in0=uf[:], in1=cview_flat(7), op=AL.min)
                nc.vector.tensor_copy(out=u16[:], in_=uf[:])

                # attw = softmax(logits_h) over the 16 (level, point) slots
                lg = bap(logit_all, [[3072, 128], [32, NCH], [1, 16]], off=h * 16)
                mx = cpool.tile([128, NCH], f32, tag="mx")
                nc.vector.tensor_reduce(out=mx[:], in_=lg, axis=mybir.AxisListType.X,
                                        op=AL.max)
                es = cpool.tile([128, NCH, 16], f32, tag="tB")
                nc.vector.tensor_tensor(
                    out=es[:], in0=lg,
                    in1=bap(mx, [[96, 128], [1, NCH], [0, 16]]), op=AL.subtract)
                nc.scalar.activation(out=es[:], in_=es[:], func=AF.Exp)
                sm = cpool.tile([128, NCH], f32, tag="mx2")
                nc.vector.tensor_reduce(out=sm[:], in_=es[:], axis=mybir.AxisListType.X,
                                        op=AL.add)
                rc = cpool.tile([128, NCH], f32, tag="rc")
                nc.vector.reciprocal(out=rc[:], in_=sm[:])
                at = cpool.tile([128, NCH, 16], f32, tag="at")
                nc.vector.tensor_tensor(
                    out=at[:], in0=es[:],
                    in1=bap(rc, [[96, 128], [1, NCH], [0, 16]]), op=AL.mult)

                # corner weights w4[(c, lp, corner)]
                (wxl, wxr), (wyt, wyb) = wghts
                wta = cpool.tile([128, NCH, 16], f32, tag="tA")
                nc.vector.tensor_tensor(out=wta[:], in0=wyt[:], in1=at[:], op=AL.mult)
                wba = cpool.tile([128, NCH, 16], f32, tag="tB")
                nc.vector.tensor_tensor(out=wba[:], in0=wyb[:], in1=at[:], op=AL.mult)
                for corner, (wx, wy) in enumerate(
                        ((wxl, wta), (wxr, wta), (wxl, wba), (wxr, wba))):
                    nc.vector.tensor_tensor(
                        out=bap(w4, [[6144, 128], [64, NCH], [4, 16]], off=corner),
                        in0=wx[:], in1=wy[:], op=AL.mult)

            # ------------ phase D: wrapped index layout ------------
            with tc.tile_pool(name="phD", bufs=1) as dpool:
                idxw = idx_p.tile([128, NCH * 128], i16, tag="idxw")
                Bt = dpool.tile([16, 8, NCH * 16], i16, tag="Bt")
                for g in range(8):
                    nc.sync.dma_start(out=Bt[:, g, :],
                                      in_=bap(u16[16 * g:16 * (g + 1), :, :],
                                              [[NCH * 16, 16], [1, NCH * 16]]))
                nc.vector.tensor_copy(
                    out=bap(idxw, [[NCH * 128, 16], [128, NCH], [8, 16], [1, 8]]),
                    in_=bap(Bt, [[8 * NCH * 16, 16], [16, NCH], [1, 16], [NCH * 16, 8]]))
                for g in range(1, 8):
                    nc.sync.dma_start(out=idxw[16 * g:16 * (g + 1), :],
                                      in_=idxw[0:16, :])

            # ------------ phase E: gather, weight, reduce ------------
            # SWDGE caps one dma_gather at 1024 indices (64 descriptors per
            # SDMA engine); issue 2*G calls of 1024 per chunk group.
            with tc.tile_pool(name="phE", bufs=2) as epool:
                for cg in range(NCG):
                    g_t = epool.tile([128, G * 16, 128], f32, tag="gt")
                    for k in range(2 * G):
                        nc.gpsimd.dma_gather(
                            out_ap=g_t[:, k * 8:(k + 1) * 8, :], in_ap=qt[h][:],
                            idxs_ap=idxw[:, cg * (NI // 16) + k * 64:
                                         cg * (NI // 16) + (k + 1) * 64],
                            num_idxs=1024, num_idxs_reg=1024, elem_size=128)
                    nc.vector.tensor_tensor(
                        out=bap(g_t, [[NI, 128], [2048, G], [32, 64], [1, 32]]),
                        in0=bap(g_t, [[NI, 128], [2048, G], [32, 64], [1, 32]]),
                        in1=bap(w4, [[6144, 128], [64, G], [1, 64], [0, 32]],
                                off=cg * G * 64),
                        op=AL.mult)
                    nc.vector.tensor_reduce(
                        out=sampled[h][:, cg * G:(cg + 1) * G, :],
                        in_=bap(g_t, [[NI, 128], [2048, G], [1, 32], [32, 64]]),
                        axis=mybir.AxisListType.X, op=AL.add)

        # ---------------- phase F: output projection ----------------
        with tc.tile_pool(name="phF", bufs=3) as fpool, \
             tc.tile_pool(name="phF_ps", bufs=3, space="PSUM") as fps:
            for c in range(NCH):
                sT = fpool.tile([64, 128], f32, tag="sT")
                for h in range(2):
                    tp = fps.tile([32, 128], f32, tag="tp")
                    nc.tensor.transpose(out=tp[:], in_=sampled[h][:, c, :],
                                        identity=ident[:])
                    nc.scalar.copy(out=sT[h * 32:(h + 1) * 32, :], in_=tp[:])
                po = fps.tile([128, DM], f32, tag="po")
                nc.tensor.matmul(out=po[:], lhsT=sT[:], rhs=wo_t[:],
                                 start=True, stop=True)
                ob = fpool.tile([128, DM], f32, tag="ob")
                nc.scalar.copy(out=ob[:], in_=po[:])
                nc.sync.dma_start(out=out_d[c * 128:(c + 1) * 128, :], in_=ob[:])

    nc.finalize()
    if do_split:
        split_waits(nc)
    return nc


def _get_nc():
    if "nc" not in _NC_CACHE:
        _NC_CACHE["nc"] = build_nc()
    return _NC_CACHE["nc"]


def make_in_maps(query, reference_points, input_flatten, spatial_shapes,
                 Wv, bv, Ws, bs, Wa, ba, Wo, bo):
    consts = _consts_array()
    in_maps = []
    for core in range(8):
        b = core // 4
        m0 = 2 * (core % 4)
        def pad(a):
            a = np.asarray(a, np.float32)
            p = np.zeros((LQP, a.shape[1]), np.float32)
            p[:LQ] = a
            return p

        in_maps.append({
            "x": pad(input_flatten[b]),
            "q": pad(query[b]),
            "ref": pad(np.asarray(reference_points[b]).reshape(LQ, 8)),
            "wv": np.ascontiguousarray(Wv[:, 32 * m0:32 * m0 + 64], np.float32),
            "ws": np.ascontiguousarray(Ws[:, 32 * m0:32 * m0 + 64], np.float32),
            "wa": np.ascontiguousarray(Wa[:, 16 * m0:16 * m0 + 32], np.float32),
            "wo": np.ascontiguousarray(Wo[32 * m0:32 * m0 + 64, :], np.float32),
            "bs": np.ascontiguousarray(bs[32 * m0:32 * m0 + 64], np.float32),
            "bv": np.ascontiguousarray(bv[32 * m0:32 * m0 + 64], np.float32),
            "ba": np.ascontiguousarray(ba[16 * m0:16 * m0 + 32], np.float32),
            "consts": consts,
        })
    return in_maps


def kernel(query, reference_points, input_flatten, spatial_shapes,
           Wv, bv, Ws, bs, Wa, ba, Wo, bo, _trace=False, _trace_kwargs=None):
    args = [np.asarray(a) for a in (
        query, reference_points, input_flatten, spatial_shapes,
        Wv, bv, Ws, bs, Wa, ba, Wo, bo)]
    nc = _get_nc()
    in_maps = make_in_maps(*args)
    kw = {}
    if _trace:
        kw["trace"] = True
        if _trace_kwargs:
            kw.update(_trace_kwargs)
    res = run_bass_kernel_spmd(nc, in_maps, core_ids=list(range(8)), **kw)
    out = np.zeros((B, LQ, DM), np.float32)
    for core in range(8):
        out[core // 4] += res.results[core]["out"][:LQ]
    out += np.asarray(bo, np.float32)[None, None, :]
    if _trace:
        return out, res
    return out

